# revision 11
# baseline (speedup 1.0000x reference)
"""Trainium2 Bass kernel for nn_DGL_Net (3-layer GraphConv GNN, 50000 nodes, 800k edges).

Strategy (8 NeuronCores, SPMD):
  - Host: relabel nodes into 392 balanced tiles of 128 nodes (<=2047 in-edges per
    tile), 49 tiles per core, rows numbered chunk-major (4 chunks per core) so
    AllGathers can be chunked and overlapped with compute. Per layer: local
    matmul (bf16) -> scale by c_src -> chunked AllGather of per-node activations
    (overlapped with the producing loop) -> per-tile dma_gather (2048 idxs, 4
    SWDGE queues) -> one-hot (Sel) matmul aggregation in PSUM -> scale by c_dst
    + bias (+relu / per-tile log_softmax).
  - Sel[e,d] = (dst_local[e] == d) is built on the vector engine from an iota
    row constant and a per-chunk dst-lane column (is_equal), zero DMA traffic.
    Dummy (padding) slots carry dst_local=-1 so their Sel column is all-zero.
  - int16 gather indices: gather base is offset +32768 rows so idx = row-32768
    spans the whole [0, 50176) row space within int16. The last slot of every
    2048-index gather call is a reserved dummy with idx>=0 (defeats the ucode's
    trailing-negative trim).
"""
import os
import sys

sys.path.insert(0, '/opt/trn_rl_repo')

import numpy as np
import ml_dtypes

import concourse.bass as bass
import concourse.bacc as bacc
import concourse.mybir as mybir
import concourse.tile as tile
from concourse.bass_utils import run_bass_kernel_spmd

BF16 = ml_dtypes.bfloat16

N_NODES = 50000
N_CORES = 8
TILE_N = 128                 # nodes per tile
TILES_PER_CORE = 49
N_TILES = N_CORES * TILES_PER_CORE      # 392
ROWS_PER_CORE = TILES_PER_CORE * TILE_N  # 6272
N_ROWS = N_CORES * ROWS_PER_CORE         # 50176
R_CHUNKS = 16                # edge chunks (of 128 slots) per tile
SLOTS_PER_TILE = R_CHUNKS * 128          # 2048
SLOTS = TILES_PER_CORE * SLOTS_PER_TILE  # 100352 per core
CALL = int(os.environ.get('BASS_CALL', '1024'))  # idxs per dma_gather call
N_CALLS = SLOTS // CALL
CALLS_PER_TILE = SLOTS_PER_TILE // CALL if CALL <= SLOTS_PER_TILE else 1
RES_PER_TILE = max(SLOTS_PER_TILE // CALL, 1)    # reserved call-end dummies
TILE_EDGE_CAP = SLOTS_PER_TILE - RES_PER_TILE
CHUNKS = TILES_PER_CORE * R_CHUNKS       # 784 chunks per core
IDX_OFF = 32768              # gather base offset (int16 trick)
F_IN = 1433
F_IN_P = 1536                # padded to 12*128
KC1 = F_IN_P // 128          # 12
F1 = 256
F2 = 32
F3 = 7
FPAD = 128                   # padded row width for M2/M3 gather (256B elems)

# AllGather chunking: tiles grouped into 4 chunks; global row numbering is
# chunk-major (chunk, core, tile-in-chunk, lane) so each chunked AllGather
# writes a contiguous slice of m_full.
CH_STARTS = [0, 16, 28, 40]
CH_TILES = [16, 12, 12, 9]
CH_ROWS = [t * TILE_N for t in CH_TILES]          # [2048,1536,1536,1152]
CH_BASE = [0, 16384, 28672, 40960]                # global row base per chunk
CH_ENDS = [15, 27, 39, 48]                        # last tile of each chunk

AG_CHUNKED = os.environ.get('AG_CHUNKED', '1') == '1'

last_exec_time_ns = None


def _chunk_of_tile(t):
    for k in range(3, -1, -1):
        if t >= CH_STARTS[k]:
            return k
    raise AssertionError


def _global_row(c, t, lane):
    k = _chunk_of_tile(t)
    tt = t - CH_STARTS[k]
    return CH_BASE[k] + c * CH_ROWS[k] + tt * TILE_N + lane


def _preprocess(edge_index):
    """Graph preprocessing: normalization constants, node->($core,tile,lane)
    relabeling with balanced per-tile in-degree, per-core edge slot tables."""
    src = np.asarray(edge_index[0], dtype=np.int64)
    dst = np.asarray(edge_index[1], dtype=np.int64)
    n_edges = src.shape[0]

    deg_out = np.bincount(src, minlength=N_NODES).astype(np.float64)
    deg_in = np.bincount(dst, minlength=N_NODES).astype(np.float64)
    c_src = (1.0 / np.sqrt(np.maximum(deg_out, 1.0))).astype(np.float32)
    c_dst = (1.0 / np.sqrt(np.maximum(deg_in, 1.0))).astype(np.float32)

    # --- greedy balanced tile packing by in-degree ---
    import heapq
    order = np.argsort(-deg_in, kind='stable')
    heap = [(0.0, 0, t) for t in range(N_TILES)]  # (load, count, tile)
    heapq.heapify(heap)
    tile_nodes = [[] for _ in range(N_TILES)]
    tile_load = np.zeros(N_TILES)
    deferred = []
    for v in order:
        dv = deg_in[v]
        while True:
            load, cnt, t = heapq.heappop(heap)
            if cnt >= TILE_N:
                continue  # stale/full
            if load + dv > TILE_EDGE_CAP:
                deferred.append((load, cnt, t))
                continue
            break
        tile_nodes[t].append(int(v))
        tile_load[t] = load + dv
        heapq.heappush(heap, (load + dv, cnt + 1, t))
        for item in deferred:
            heapq.heappush(heap, item)
        deferred = []
    assert max(tile_load) <= TILE_EDGE_CAP

    # sort tiles by load desc, group by 8, core c takes c-th of each group
    tsort = np.argsort(-tile_load, kind='stable')
    tile_assign = np.empty((N_CORES, TILES_PER_CORE), dtype=np.int64)
    for k in range(TILES_PER_CORE):
        for c in range(N_CORES):
            tile_assign[c, k] = tsort[k * N_CORES + c]

    # row mapping (chunk-major global rows)
    row_of_node = np.full(N_NODES, -1, dtype=np.int64)
    node_of_row = np.full(N_ROWS, -1, dtype=np.int64)  # -1 = virtual pad node
    own_node = np.full((N_CORES, ROWS_PER_CORE), -1, dtype=np.int64)
    for c in range(N_CORES):
        for k in range(TILES_PER_CORE):
            t = tile_assign[c, k]
            nodes = tile_nodes[t]
            for lane, v in enumerate(nodes):
                g = _global_row(c, k, lane)
                row_of_node[v] = g
                node_of_row[g] = v
                own_node[c, k * TILE_N + lane] = v
    assert (row_of_node >= 0).all()

    # --- per-core edge slot tables ---
    dst_row = row_of_node[dst]      # global rows
    src_row = row_of_node[src]
    # recover (core, tile, lane) of dst from global row
    e_core = np.empty(n_edges, dtype=np.int64)
    e_tile = np.empty(n_edges, dtype=np.int64)
    e_lane = dst_row % TILE_N
    for k in range(4):
        lo = CH_BASE[k]
        hi = CH_BASE[k] + N_CORES * CH_ROWS[k]
        m = (dst_row >= lo) & (dst_row < hi)
        rel = dst_row[m] - lo
        e_core[m] = rel // CH_ROWS[k]
        e_tile[m] = CH_STARTS[k] + (rel % CH_ROWS[k]) // TILE_N

    idx_flat = np.zeros((N_CORES, SLOTS), dtype=np.int16)      # pad idx = 0
    dst_flat = np.full((N_CORES, SLOTS), -1, dtype=np.int16)   # pad dst = -1

    # group edges by (core, tile) and assign slot positions
    key = e_core * TILES_PER_CORE + e_tile
    eorder = np.argsort(key, kind='stable')
    key_s = key[eorder]
    grp_start = np.searchsorted(key_s, np.arange(N_CORES * TILES_PER_CORE))
    pos_in_grp = np.arange(n_edges) - grp_start[key_s]
    assert pos_in_grp.max() < TILE_EDGE_CAP
    # skip the reserved last slot of each CALL-sized block within the tile
    j = pos_in_grp
    slot_in_tile = j + j // (CALL - 1) if CALL < SLOTS_PER_TILE else j
    if CALL < SLOTS_PER_TILE:
        # j -> j + number of reserved slots passed; reserved at CALL-1, 2*CALL-1, ...
        slot_in_tile = j + (j // (CALL - 1))
    assert slot_in_tile.max() < SLOTS_PER_TILE - (1 if CALL >= SLOTS_PER_TILE else 0)
    slots_abs = key_s % TILES_PER_CORE * SLOTS_PER_TILE + slot_in_tile
    cores_s = key_s // TILES_PER_CORE
    idx_flat[cores_s, slots_abs] = (src_row[eorder] - IDX_OFF).astype(np.int16)
    dst_flat[cores_s, slots_abs] = e_lane[eorder].astype(np.int16)

    # wrap idx to [128, SLOTS/16] (idx i -> [i%16 replicated, i//16])
    cols = SLOTS // 16
    idx_tile = np.zeros((N_CORES, 128, cols), dtype=np.int16)
    for c in range(N_CORES):
        w = idx_flat[c].reshape(cols, 16).T  # [16, cols]
        idx_tile[c] = np.tile(w, (8, 1))

    # per-chunk dst lane columns: dstp[c][e, ch] = dst lane of slot e in chunk ch
    dstp = np.empty((N_CORES, 128, CHUNKS), dtype=np.float32)
    for c in range(N_CORES):
        dstp[c] = dst_flat[c].reshape(CHUNKS, 128).T.astype(np.float32)

    # per-core normalization tables
    cd_row = np.where(node_of_row >= 0, c_dst[np.maximum(node_of_row, 0)], 1.0)
    cs_row = np.where(node_of_row >= 0, c_src[np.maximum(node_of_row, 0)], 1.0)
    # own-row (tile-major) order per core
    cd_own = np.empty((N_CORES, ROWS_PER_CORE), dtype=np.float32)
    cs_own = np.empty((N_CORES, ROWS_PER_CORE), dtype=np.float32)
    for c in range(N_CORES):
        for t in range(TILES_PER_CORE):
            for lane in range(TILE_N):
                g = _global_row(c, t, lane)
                cd_own[c, t * TILE_N + lane] = cd_row[g]
                cs_own[c, t * TILE_N + lane] = cs_row[g]
    cdst_rep = np.repeat(cd_own[:, None, :], 128, axis=1)  # [C,128,6272]
    cdst_pp = cd_own.reshape(N_CORES, TILES_PER_CORE, 128).transpose(0, 2, 1).copy()
    csrc_t = cs_own.reshape(N_CORES, TILES_PER_CORE, 128).transpose(0, 2, 1).copy()

    return dict(row_of_node=row_of_node, node_of_row=node_of_row,
                own_node=own_node,
                idx_tile=idx_tile, dstp=dstp,
                cdst_rep=cdst_rep.astype(np.float32), cdst_pp=cdst_pp,
                csrc_t=csrc_t)


def _build_nc():
    nc = bacc.Bacc("TRN2", target_bir_lowering=False, debug=False,
                   enable_asserts=True, num_devices=N_CORES, num_swdge_queues=4)
    dt = mybir.dt
    inp = {}
    inp['xT'] = nc.dram_tensor("xT", [F_IN_P, ROWS_PER_CORE], dt.bfloat16, kind="ExternalInput")
    inp['W1'] = nc.dram_tensor("W1", [F_IN_P, F1], dt.bfloat16, kind="ExternalInput")
    inp['W2'] = nc.dram_tensor("W2", [F1, F2], dt.bfloat16, kind="ExternalInput")
    inp['W3'] = nc.dram_tensor("W3", [F2, F3], dt.bfloat16, kind="ExternalInput")
    inp['idx'] = nc.dram_tensor("idx", [128, SLOTS // 16], dt.int16, kind="ExternalInput")
    inp['dstp'] = nc.dram_tensor("dstp", [128, CHUNKS], dt.float32, kind="ExternalInput")
    inp['iota'] = nc.dram_tensor("iota", [128, 128], dt.float32, kind="ExternalInput")
    inp['cdst_rep'] = nc.dram_tensor("cdst_rep", [128, ROWS_PER_CORE], dt.float32, kind="ExternalInput")
    inp['cdst_pp'] = nc.dram_tensor("cdst_pp", [128, TILES_PER_CORE], dt.float32, kind="ExternalInput")
    inp['csrc_t'] = nc.dram_tensor("csrc_t", [128, TILES_PER_CORE], dt.float32, kind="ExternalInput")
    inp['b1pp'] = nc.dram_tensor("b1pp", [128, 2], dt.float32, kind="ExternalInput")
    inp['b2pp'] = nc.dram_tensor("b2pp", [128, 1], dt.float32, kind="ExternalInput")
    inp['b3t'] = nc.dram_tensor("b3t", [128, F3], dt.float32, kind="ExternalInput")
    out_t = nc.dram_tensor("out", [ROWS_PER_CORE, F3], dt.float32, kind="ExternalOutput")

    m1_own = nc.dram_tensor("m1_own", [ROWS_PER_CORE, F1], dt.bfloat16)
    m1_full = nc.dram_tensor("m1_full", [N_ROWS, F1], dt.bfloat16, addr_space="Shared")
    m2_own = nc.dram_tensor("m2_own", [ROWS_PER_CORE, FPAD], dt.bfloat16)
    m2_full = nc.dram_tensor("m2_full", [N_ROWS, FPAD], dt.bfloat16, addr_space="Shared")
    m3_own = nc.dram_tensor("m3_own", [ROWS_PER_CORE, FPAD], dt.bfloat16)
    m3_full = nc.dram_tensor("m3_full", [N_ROWS, FPAD], dt.bfloat16, addr_space="Shared")

    AL = mybir.AluOpType
    AF = mybir.ActivationFunctionType
    RG = [list(range(N_CORES))]

    def ag_chunk(m_own, m_full, k):
        if not AG_CHUNKED:
            return
        a = CH_STARTS[k] * TILE_N
        b = a + CH_ROWS[k]
        ga = CH_BASE[k]
        gb = ga + N_CORES * CH_ROWS[k]
        nc.gpsimd.collective_compute(
            "AllGather", AL.bypass, replica_groups=RG,
            ins=[m_own[a:b, :]], outs=[m_full[ga:gb, :]])

    def ag_full(m_own, m_full):
        if AG_CHUNKED:
            return
        for k in range(4):
            a = CH_STARTS[k] * TILE_N
            b = a + CH_ROWS[k]
            ga = CH_BASE[k]
            gb = ga + N_CORES * CH_ROWS[k]
            nc.gpsimd.collective_compute(
                "AllGather", AL.bypass, replica_groups=RG,
                ins=[m_own[a:b, :]], outs=[m_full[ga:gb, :]])

    with tile.TileContext(nc) as tc:
        with tc.tile_pool(name="const", bufs=1) as constp, \
             tc.tile_pool(name="big", bufs=1) as bigp, \
             tc.tile_pool(name="xstream", bufs=2) as xp, \
             tc.tile_pool(name="work", bufs=3) as wp, \
             tc.tile_pool(name="gpool", bufs=4) as gp, \
             tc.tile_pool(name="selp", bufs=3) as selp, \
             tc.tile_pool(name="psA", bufs=4, space="PSUM") as psA, \
             tc.tile_pool(name="psB", bufs=2, space="PSUM") as psB, \
             tc.tile_pool(name="psmm", bufs=2, space="PSUM") as psmm:

            # ---- resident constants ----
            w1_t = constp.tile([128, KC1, F1], mybir.dt.bfloat16)
            nc.sync.dma_start(w1_t[:], inp['W1'].rearrange("(kc p) n -> p kc n", p=128))
            w2_t = constp.tile([128, 2, F2], mybir.dt.bfloat16)
            nc.sync.dma_start(w2_t[:], inp['W2'].rearrange("(kc p) n -> p kc n", p=128))
            w3_t = constp.tile([F2, F3], mybir.dt.bfloat16)
            nc.sync.dma_start(w3_t[:], inp['W3'][:, :])
            idx_t = constp.tile([128, SLOTS // 16], mybir.dt.int16)
            nc.sync.dma_start(idx_t[:], inp['idx'][:, :])
            dstp_t = constp.tile([128, CHUNKS], mybir.dt.float32)
            nc.sync.dma_start(dstp_t[:], inp['dstp'][:, :])
            iota_t = constp.tile([128, 128], mybir.dt.float32)
            nc.sync.dma_start(iota_t[:], inp['iota'][:, :])

            cdrep_t = constp.tile([128, ROWS_PER_CORE], mybir.dt.float32)
            nc.sync.dma_start(cdrep_t[:], inp['cdst_rep'][:, :])
            cdpp_t = constp.tile([128, TILES_PER_CORE], mybir.dt.float32)
            nc.sync.dma_start(cdpp_t[:], inp['cdst_pp'][:, :])
            cs_t = constp.tile([128, TILES_PER_CORE], mybir.dt.float32)
            nc.sync.dma_start(cs_t[:], inp['csrc_t'][:, :])
            b1_t = constp.tile([128, 2], mybir.dt.float32)
            nc.sync.dma_start(b1_t[:], inp['b1pp'][:, :])
            b2_t = constp.tile([128, 1], mybir.dt.float32)
            nc.sync.dma_start(b2_t[:], inp['b2pp'][:, :])
            b3_t = constp.tile([128, F3], mybir.dt.float32)
            nc.sync.dma_start(b3_t[:], inp['b3t'][:, :])

            h1t = bigp.tile([128, 2, ROWS_PER_CORE], mybir.dt.bfloat16)  # H1.T
            h2t = bigp.tile([F2, ROWS_PER_CORE], mybir.dt.bfloat16)      # H2.T

            # ---- phase 1: M1 = (X @ W1) * c_src, AG1 chunks interleaved ----
            blocks = [(i * 512, 512) for i in range(12)] + [(6144, 128)]
            for c0, bs in blocks:
                xt = xp.tile([128, KC1, bs], mybir.dt.bfloat16, tag="xt")
                nc.sync.dma_start(
                    xt[:, :, :bs],
                    inp['xT'][:, c0:c0 + bs].rearrange("(kc p) n -> p kc n", p=128))
                for sub in range(bs // 128):
                    t_idx = (c0 + sub * 128) // 128
                    ps = psmm.tile([128, F1], mybir.dt.float32, tag="mm1")
                    for kc in range(KC1):
                        nc.tensor.matmul(ps[:], xt[:, kc, sub * 128:(sub + 1) * 128],
                                         w1_t[:, kc, :], start=(kc == 0), stop=(kc == KC1 - 1))
                    ob = wp.tile([128, F1], mybir.dt.bfloat16, tag="m1o")
                    nc.vector.tensor_scalar(ob[:], ps[:], cs_t[:, t_idx:t_idx + 1], None, AL.mult)
                    nc.sync.dma_start(m1_own[t_idx * 128:(t_idx + 1) * 128, :], ob[:])
                    if t_idx in CH_ENDS:
                        ag_chunk(m1_own, m1_full, CH_ENDS.index(t_idx))

            ag_full(m1_own, m1_full)

            # ---- agg helper ----
            JPC = CALL // 128   # chunks per gather call
            def agg_layer(m_full_t, elem, consume_chunk, finish_tile):
                cur = {}
                for call in range(N_CALLS):
                    g = gp.tile([128, JPC, elem], mybir.dt.bfloat16, tag=f"g{elem}")
                    nc.gpsimd.dma_gather(
                        g[:], m_full_t[IDX_OFF:, :],
                        idx_t[:, call * (CALL // 16):(call + 1) * (CALL // 16)],
                        CALL, CALL, elem, queue_num=call % 4)
                    selt = selp.tile([128, JPC, 128], mybir.dt.bfloat16, tag="selt")
                    for j in range(JPC):
                        ch = call * JPC + j
                        nc.vector.tensor_scalar(selt[:, j, :], iota_t[:],
                                                dstp_t[:, ch:ch + 1], None, AL.is_equal)
                    for j in range(JPC):
                        ch = call * JPC + j
                        t_idx = ch // R_CHUNKS
                        cj = ch % R_CHUNKS
                        consume_chunk(cur, g, j, selt[:, j, :], t_idx,
                                      cj == 0, cj == R_CHUNKS - 1)
                        if cj == R_CHUNKS - 1:
                            finish_tile(cur, t_idx)
                            cur = {}

            # ---- layer 1 aggregation -> H1T, M2 + AG2 chunks inline ----
            def l1_chunk(cur, g, j, sel, t_idx, first, last):
                if first:
                    cur[0] = psA.tile([128, 128], mybir.dt.float32, tag="aggA", name="psa1")
                    cur[1] = psB.tile([128, 128], mybir.dt.float32, tag="aggB", name="psb1")
                for fc in range(2):
                    nc.tensor.matmul(cur[fc][:], g[:, j, fc * 128:(fc + 1) * 128],
                                     sel, start=first, stop=last)

            def l1_tile(cur, t_idx):
                sl = slice(t_idx * 128, (t_idx + 1) * 128)
                for fc in range(2):
                    nc.vector.tensor_tensor(h1t[:, fc, sl], cur[fc][:],
                                            cdrep_t[:, sl], AL.mult)
                    nc.scalar.activation(h1t[:, fc, sl], h1t[:, fc, sl],
                                         AF.Relu, bias=b1_t[:, fc:fc + 1])
                # M2 tile inline
                ps = psmm.tile([128, F2], mybir.dt.float32, tag="mm1")
                for fc in range(2):
                    nc.tensor.matmul(ps[:], h1t[:, fc, sl], w2_t[:, fc, :],
                                     start=(fc == 0), stop=(fc == 1))
                ob = wp.tile([128, FPAD], mybir.dt.bfloat16, tag="m2o")
                nc.vector.tensor_scalar(ob[:, 0:F2], ps[:], cs_t[:, t_idx:t_idx + 1], None, AL.mult)
                nc.sync.dma_start(m2_own[t_idx * 128:(t_idx + 1) * 128, :], ob[:])
                if t_idx in CH_ENDS:
                    ag_chunk(m2_own, m2_full, CH_ENDS.index(t_idx))

            agg_layer(m1_full, F1, l1_chunk, l1_tile)
            ag_full(m2_own, m2_full)

            # ---- layer 2 aggregation -> H2T, M3 + AG3 chunks inline ----
            def l2_chunk(cur, g, j, sel, t_idx, first, last):
                if first:
                    cur[0] = psA.tile([F2, 128], mybir.dt.float32, tag="aggA", name="psa2")
                nc.tensor.matmul(cur[0][:], g[:, j, 0:F2], sel, start=first, stop=last)

            def l2_tile(cur, t_idx):
                sl = slice(t_idx * 128, (t_idx + 1) * 128)
                nc.vector.tensor_tensor(h2t[:, sl], cur[0][:], cdrep_t[0:F2, sl], AL.mult)
                nc.scalar.activation(h2t[:, sl], h2t[:, sl], AF.Relu, bias=b2_t[0:F2, 0:1])
                # M3 tile inline
                ps = psmm.tile([128, F3], mybir.dt.float32, tag="mm1")
                nc.tensor.matmul(ps[:], h2t[:, sl], w3_t[:], start=True, stop=True)
                ob = wp.tile([128, FPAD], mybir.dt.bfloat16, tag="m3o")
                nc.vector.tensor_scalar(ob[:, 0:F3], ps[:], cs_t[:, t_idx:t_idx + 1], None, AL.mult)
                nc.sync.dma_start(m3_own[t_idx * 128:(t_idx + 1) * 128, :], ob[:])
                if t_idx in CH_ENDS:
                    ag_chunk(m3_own, m3_full, CH_ENDS.index(t_idx))

            agg_layer(m2_full, FPAD, l2_chunk, l2_tile)
            ag_full(m3_own, m3_full)

            # ---- layer 3 aggregation -> per-tile log_softmax -> out ----
            def l3_chunk(cur, g, j, sel, t_idx, first, last):
                if first:
                    cur[0] = psA.tile([128, F3], mybir.dt.float32, tag="aggA", name="psa3")
                nc.tensor.matmul(cur[0][:], sel, g[:, j, 0:F3], start=first, stop=last)

            def l3_tile(cur, t_idx):
                x3 = wp.tile([128, F3], mybir.dt.float32, tag="x3", name="x3")
                nc.vector.tensor_scalar(x3[:], cur[0][:], cdpp_t[:, t_idx:t_idx + 1], None, AL.mult)
                nc.vector.tensor_tensor(x3[:], x3[:], b3_t[:], AL.add)
                ex = wp.tile([128, F3], mybir.dt.float32, tag="ex", name="ex")
                nc.scalar.activation(ex[:], x3[:], AF.Exp)
                sm = wp.tile([128, 1], mybir.dt.float32, tag="sm", name="sm")
                nc.vector.tensor_reduce(
                    sm[:], ex[:].rearrange("p (t f) -> p t f", f=F3),
                    mybir.AxisListType.X, AL.add)
                nl = wp.tile([128, 1], mybir.dt.float32, tag="nl", name="nl")
                nc.scalar.activation(nl[:], sm[:], AF.Ln)
                ox = wp.tile([128, F3], mybir.dt.float32, tag="ox", name="ox")
                nc.vector.tensor_scalar(ox[:], x3[:], nl[:, 0:1], None, AL.subtract)
                nc.sync.dma_start(out_t[t_idx * 128:(t_idx + 1) * 128, :], ox[:])

            agg_layer(m3_full, FPAD, l3_chunk, l3_tile)

    nc.compile()
    return nc


def _install_profile_shim():
    """Provide the missing antenv.axon_hooks module so trace=True works under axon."""
    try:
        import types
        import antenv
        if 'antenv.axon_hooks' in sys.modules:
            return
        _hook = [None]
        mod = types.ModuleType('antenv.axon_hooks')
        mod.set_axon_ntff_profile_hook = lambda h: _hook.__setitem__(0, h)
        mod.get_axon_ntff_profile_hook = lambda: _hook[0]
        sys.modules['antenv.axon_hooks'] = mod
        antenv.axon_hooks = mod
        from trn_agent_boot.trn_boot import _ntff_profile_via_ctypes
        mod.set_axon_ntff_profile_hook(
            _ntff_profile_via_ctypes('/opt/axon/libaxon_pjrt.so'))
    except Exception:
        pass


_CACHE = {}


def kernel(features, edge_index, W1, b1, W2, b2, W3, b3):
    global last_exec_time_ns
    features = np.asarray(features, dtype=np.float32)
    pre = _preprocess(np.asarray(edge_index))

    if 'nc' not in _CACHE:
        _CACHE['nc'] = _build_nc()
    nc = _CACHE['nc']

    # host-side input prep
    W1p = np.zeros((F_IN_P, F1), dtype=BF16)
    W1p[:F_IN] = np.asarray(W1, dtype=BF16)
    W2b = np.asarray(W2, dtype=BF16)
    W3b = np.asarray(W3, dtype=BF16)
    b1pp = np.asarray(b1, dtype=np.float32).reshape(2, 128).T.copy()
    b2pp = np.zeros((128, 1), dtype=np.float32)
    b2pp[:F2, 0] = np.asarray(b2, dtype=np.float32)
    b3t = np.tile(np.asarray(b3, dtype=np.float32), (128, 1))
    iota_bf = np.tile(np.arange(128, dtype=np.float32), (128, 1))

    # features, permuted and transposed per core: [F_IN_P, 6272] bf16
    feat_b = features.astype(BF16)
    in_maps = []
    for c in range(N_CORES):
        rows = pre['own_node'][c]
        xTc = np.zeros((F_IN_P, ROWS_PER_CORE), dtype=BF16)
        real = rows >= 0
        xTc[:F_IN, real] = feat_b[rows[real]].T
        in_maps.append({
            'xT': xTc, 'W1': W1p, 'W2': W2b, 'W3': W3b,
            'idx': pre['idx_tile'][c], 'dstp': pre['dstp'][c], 'iota': iota_bf,
            'cdst_rep': pre['cdst_rep'][c], 'cdst_pp': pre['cdst_pp'][c],
            'csrc_t': pre['csrc_t'][c],
            'b1pp': b1pp, 'b2pp': b2pp, 'b3t': b3t,
        })

    trace = os.environ.get('BASS_KERNEL_TRACE', '0') == '1'
    if trace:
        _install_profile_shim()
    res = run_bass_kernel_spmd(nc, in_maps, core_ids=list(range(N_CORES)), trace=trace)
    last_exec_time_ns = res.exec_time_ns

    # assemble + inverse permute (own rows are tile-major per core)
    out = np.empty((N_NODES, F3), dtype=np.float32)
    for c in range(N_CORES):
        rows = pre['own_node'][c]
        real = rows >= 0
        out[rows[real]] = res.results[c]['out'][real]
    return out


# revision 15
# speedup vs baseline: 1.5202x; 1.5202x over previous
"""Trainium2 Bass kernel for nn_DGL_Net (3-layer GraphConv GNN, 50000 nodes, 800k edges).

Strategy (8 NeuronCores, SPMD):
  - Host: relabel nodes into 392 balanced tiles of 128 nodes (<=2047 in-edges per
    tile), 49 tiles per core, rows numbered chunk-major (4 chunks per core) so
    AllGathers can be chunked and overlapped with compute. Per layer: local
    matmul (bf16) -> scale by c_src -> chunked AllGather of per-node activations
    (overlapped with the producing loop) -> per-tile dma_gather (2048 idxs, 4
    SWDGE queues) -> one-hot (Sel) matmul aggregation in PSUM -> scale by c_dst
    + bias (+relu / per-tile log_softmax).
  - Sel[e,d] = (dst_local[e] == d) is built on the vector engine from an iota
    row constant and a per-chunk dst-lane column (is_equal), zero DMA traffic.
    Dummy (padding) slots carry dst_local=-1 so their Sel column is all-zero.
  - int16 gather indices: gather base is offset +32768 rows so idx = row-32768
    spans the whole [0, 50176) row space within int16. The last slot of every
    2048-index gather call is a reserved dummy with idx>=0 (defeats the ucode's
    trailing-negative trim).
"""
import os
import sys

sys.path.insert(0, '/opt/trn_rl_repo')

import numpy as np
import ml_dtypes

import concourse.bass as bass
import concourse.bacc as bacc
import concourse.mybir as mybir
import concourse.tile as tile
from concourse.bass_utils import run_bass_kernel_spmd

BF16 = ml_dtypes.bfloat16

N_NODES = 50000
N_CORES = 8
TILE_N = 128                 # nodes per tile
TILES_PER_CORE = 49
N_TILES = N_CORES * TILES_PER_CORE      # 392
ROWS_PER_CORE = TILES_PER_CORE * TILE_N  # 6272
N_ROWS = N_CORES * ROWS_PER_CORE         # 50176
R_CHUNKS = 16                # edge chunks (of 128 slots) per tile
SLOTS_PER_TILE = R_CHUNKS * 128          # 2048
SLOTS = TILES_PER_CORE * SLOTS_PER_TILE  # 100352 per core
CALL = int(os.environ.get('BASS_CALL', '1024'))  # idxs per dma_gather call
N_CALLS = SLOTS // CALL
CALLS_PER_TILE = SLOTS_PER_TILE // CALL if CALL <= SLOTS_PER_TILE else 1
RES_PER_TILE = max(SLOTS_PER_TILE // CALL, 1)    # reserved call-end dummies
TILE_EDGE_CAP = SLOTS_PER_TILE - RES_PER_TILE
CHUNKS = TILES_PER_CORE * R_CHUNKS       # 784 chunks per core
IDX_OFF = 32768              # gather base offset (int16 trick)
F_IN = 1433
F_IN_P = 1536                # padded to 12*128
KC1 = F_IN_P // 128          # 12
F1 = 256
F2 = 32
F3 = 7
FPAD = 256                   # padded row width for M2/M3 gather (fp8 elems, 256B)

# AllGather chunking: tiles grouped into 4 chunks; global row numbering is
# chunk-major (chunk, core, tile-in-chunk, lane) so each chunked AllGather
# writes a contiguous slice of m_full.
CH_STARTS = [0, 16, 28, 40]
CH_TILES = [16, 12, 12, 9]
CH_ROWS = [t * TILE_N for t in CH_TILES]          # [2048,1536,1536,1152]
CH_BASE = [0, 16384, 28672, 40960]                # global row base per chunk
CH_ENDS = [15, 27, 39, 48]                        # last tile of each chunk

AG_CHUNKED = os.environ.get('AG_CHUNKED', '1') == '1'

last_exec_time_ns = None


def _chunk_of_tile(t):
    for k in range(3, -1, -1):
        if t >= CH_STARTS[k]:
            return k
    raise AssertionError


def _global_row(c, t, lane):
    k = _chunk_of_tile(t)
    tt = t - CH_STARTS[k]
    return CH_BASE[k] + c * CH_ROWS[k] + tt * TILE_N + lane


def _preprocess(edge_index):
    """Graph preprocessing: normalization constants, node->($core,tile,lane)
    relabeling with balanced per-tile in-degree, per-core edge slot tables."""
    src = np.asarray(edge_index[0], dtype=np.int64)
    dst = np.asarray(edge_index[1], dtype=np.int64)
    n_edges = src.shape[0]

    deg_out = np.bincount(src, minlength=N_NODES).astype(np.float64)
    deg_in = np.bincount(dst, minlength=N_NODES).astype(np.float64)
    c_src = (1.0 / np.sqrt(np.maximum(deg_out, 1.0))).astype(np.float32)
    c_dst = (1.0 / np.sqrt(np.maximum(deg_in, 1.0))).astype(np.float32)

    # --- greedy balanced tile packing by in-degree ---
    import heapq
    order = np.argsort(-deg_in, kind='stable')
    heap = [(0.0, 0, t) for t in range(N_TILES)]  # (load, count, tile)
    heapq.heapify(heap)
    tile_nodes = [[] for _ in range(N_TILES)]
    tile_load = np.zeros(N_TILES)
    deferred = []
    for v in order:
        dv = deg_in[v]
        while True:
            load, cnt, t = heapq.heappop(heap)
            if cnt >= TILE_N:
                continue  # stale/full
            if load + dv > TILE_EDGE_CAP:
                deferred.append((load, cnt, t))
                continue
            break
        tile_nodes[t].append(int(v))
        tile_load[t] = load + dv
        heapq.heappush(heap, (load + dv, cnt + 1, t))
        for item in deferred:
            heapq.heappush(heap, item)
        deferred = []
    assert max(tile_load) <= TILE_EDGE_CAP

    # sort tiles by load desc, group by 8, core c takes c-th of each group
    tsort = np.argsort(-tile_load, kind='stable')
    tile_assign = np.empty((N_CORES, TILES_PER_CORE), dtype=np.int64)
    for k in range(TILES_PER_CORE):
        for c in range(N_CORES):
            tile_assign[c, k] = tsort[k * N_CORES + c]

    # row mapping (chunk-major global rows)
    row_of_node = np.full(N_NODES, -1, dtype=np.int64)
    node_of_row = np.full(N_ROWS, -1, dtype=np.int64)  # -1 = virtual pad node
    own_node = np.full((N_CORES, ROWS_PER_CORE), -1, dtype=np.int64)
    for c in range(N_CORES):
        for k in range(TILES_PER_CORE):
            t = tile_assign[c, k]
            nodes = tile_nodes[t]
            for lane, v in enumerate(nodes):
                g = _global_row(c, k, lane)
                row_of_node[v] = g
                node_of_row[g] = v
                own_node[c, k * TILE_N + lane] = v
    assert (row_of_node >= 0).all()

    # --- per-core edge slot tables ---
    dst_row = row_of_node[dst]      # global rows
    src_row = row_of_node[src]
    # recover (core, tile, lane) of dst from global row
    e_core = np.empty(n_edges, dtype=np.int64)
    e_tile = np.empty(n_edges, dtype=np.int64)
    e_lane = dst_row % TILE_N
    for k in range(4):
        lo = CH_BASE[k]
        hi = CH_BASE[k] + N_CORES * CH_ROWS[k]
        m = (dst_row >= lo) & (dst_row < hi)
        rel = dst_row[m] - lo
        e_core[m] = rel // CH_ROWS[k]
        e_tile[m] = CH_STARTS[k] + (rel % CH_ROWS[k]) // TILE_N

    idx_flat = np.zeros((N_CORES, SLOTS), dtype=np.int16)      # pad idx = 0
    dst_flat = np.full((N_CORES, SLOTS), -1, dtype=np.int16)   # pad dst = -1

    # group edges by (core, tile) and assign slot positions
    key = e_core * TILES_PER_CORE + e_tile
    eorder = np.argsort(key, kind='stable')
    key_s = key[eorder]
    grp_start = np.searchsorted(key_s, np.arange(N_CORES * TILES_PER_CORE))
    pos_in_grp = np.arange(n_edges) - grp_start[key_s]
    assert pos_in_grp.max() < TILE_EDGE_CAP
    # skip the reserved last slot of each CALL-sized block within the tile
    j = pos_in_grp
    slot_in_tile = j + j // (CALL - 1) if CALL < SLOTS_PER_TILE else j
    if CALL < SLOTS_PER_TILE:
        # j -> j + number of reserved slots passed; reserved at CALL-1, 2*CALL-1, ...
        slot_in_tile = j + (j // (CALL - 1))
    assert slot_in_tile.max() < SLOTS_PER_TILE - (1 if CALL >= SLOTS_PER_TILE else 0)
    slots_abs = key_s % TILES_PER_CORE * SLOTS_PER_TILE + slot_in_tile
    cores_s = key_s // TILES_PER_CORE
    idx_flat[cores_s, slots_abs] = (src_row[eorder] - IDX_OFF).astype(np.int16)
    dst_flat[cores_s, slots_abs] = e_lane[eorder].astype(np.int16)

    # wrap idx to [128, SLOTS/16] (idx i -> [i%16 replicated, i//16])
    cols = SLOTS // 16
    idx_tile = np.zeros((N_CORES, 128, cols), dtype=np.int16)
    for c in range(N_CORES):
        w = idx_flat[c].reshape(cols, 16).T  # [16, cols]
        idx_tile[c] = np.tile(w, (8, 1))

    # one-hot Sel cache per chunk: [128e, CHUNKS*128d] fp8
    F8 = ml_dtypes.float8_e4m3
    selc = np.zeros((N_CORES, 128, CHUNKS * 128), dtype=F8)
    dr = np.arange(128, dtype=np.int16)
    for c in range(N_CORES):
        dd = dst_flat[c].reshape(CHUNKS, 128)  # [ch, e]
        oh = (dd[:, :, None] == dr[None, None, :])  # [ch, e, d]
        selc[c] = oh.transpose(1, 0, 2).reshape(128, CHUNKS * 128).astype(F8)

    # per-core normalization tables
    cd_row = np.where(node_of_row >= 0, c_dst[np.maximum(node_of_row, 0)], 1.0)
    cs_row = np.where(node_of_row >= 0, c_src[np.maximum(node_of_row, 0)], 1.0)
    # own-row (tile-major) order per core
    cd_own = np.empty((N_CORES, ROWS_PER_CORE), dtype=np.float32)
    cs_own = np.empty((N_CORES, ROWS_PER_CORE), dtype=np.float32)
    for c in range(N_CORES):
        for t in range(TILES_PER_CORE):
            for lane in range(TILE_N):
                g = _global_row(c, t, lane)
                cd_own[c, t * TILE_N + lane] = cd_row[g]
                cs_own[c, t * TILE_N + lane] = cs_row[g]
    cdst_rep = np.repeat(cd_own[:, None, :], 128, axis=1)  # [C,128,6272]
    cdst_pp = cd_own.reshape(N_CORES, TILES_PER_CORE, 128).transpose(0, 2, 1).copy()
    csrc_t = cs_own.reshape(N_CORES, TILES_PER_CORE, 128).transpose(0, 2, 1).copy()

    return dict(row_of_node=row_of_node, node_of_row=node_of_row,
                own_node=own_node,
                idx_tile=idx_tile, selc=selc,
                cdst_rep=cdst_rep.astype(np.float32), cdst_pp=cdst_pp,
                csrc_t=csrc_t)


def _build_nc():
    nc = bacc.Bacc("TRN2", target_bir_lowering=False, debug=False,
                   enable_asserts=True, num_devices=N_CORES, num_swdge_queues=4)
    dt = mybir.dt
    inp = {}
    inp['xT'] = nc.dram_tensor("xT", [F_IN_P, ROWS_PER_CORE], dt.bfloat16, kind="ExternalInput")
    inp['W1'] = nc.dram_tensor("W1", [F_IN_P, F1], dt.bfloat16, kind="ExternalInput")
    inp['W2'] = nc.dram_tensor("W2", [F1, F2], dt.bfloat16, kind="ExternalInput")
    inp['W3'] = nc.dram_tensor("W3", [F2, F3], dt.bfloat16, kind="ExternalInput")
    inp['idx'] = nc.dram_tensor("idx", [128, SLOTS // 16], dt.int16, kind="ExternalInput")
    inp['selc'] = nc.dram_tensor("selc", [128, CHUNKS * 128], dt.float8e4, kind="ExternalInput")
    inp['b1bc'] = nc.dram_tensor("b1bc", [128, F1], dt.float32, kind="ExternalInput")
    inp['ident'] = nc.dram_tensor("ident", [128, 128], dt.bfloat16, kind="ExternalInput")
    inp['cdst_rep'] = nc.dram_tensor("cdst_rep", [128, ROWS_PER_CORE], dt.float32, kind="ExternalInput")
    inp['cdst_pp'] = nc.dram_tensor("cdst_pp", [128, TILES_PER_CORE], dt.float32, kind="ExternalInput")
    inp['csrc_t'] = nc.dram_tensor("csrc_t", [128, TILES_PER_CORE], dt.float32, kind="ExternalInput")
    inp['b1pp'] = nc.dram_tensor("b1pp", [128, 2], dt.float32, kind="ExternalInput")
    inp['b2pp'] = nc.dram_tensor("b2pp", [128, 1], dt.float32, kind="ExternalInput")
    inp['b3t'] = nc.dram_tensor("b3t", [128, F3], dt.float32, kind="ExternalInput")
    out_t = nc.dram_tensor("out", [ROWS_PER_CORE, F3], dt.float32, kind="ExternalOutput")

    m1_own = nc.dram_tensor("m1_own", [ROWS_PER_CORE, F1], dt.float8e4)
    m1_full = nc.dram_tensor("m1_full", [N_ROWS, F1], dt.float8e4, addr_space="Shared")
    m2_own = nc.dram_tensor("m2_own", [ROWS_PER_CORE, FPAD], dt.float8e4)
    m2_full = nc.dram_tensor("m2_full", [N_ROWS, FPAD], dt.float8e4, addr_space="Shared")
    m3_own = nc.dram_tensor("m3_own", [ROWS_PER_CORE, FPAD], dt.float8e4)
    m3_full = nc.dram_tensor("m3_full", [N_ROWS, FPAD], dt.float8e4, addr_space="Shared")

    AL = mybir.AluOpType
    AF = mybir.ActivationFunctionType
    RG = [list(range(N_CORES))]

    def ag_chunk(m_own, m_full, k):
        if not AG_CHUNKED:
            return
        a = CH_STARTS[k] * TILE_N
        b = a + CH_ROWS[k]
        ga = CH_BASE[k]
        gb = ga + N_CORES * CH_ROWS[k]
        nc.gpsimd.collective_compute(
            "AllGather", AL.bypass, replica_groups=RG,
            ins=[m_own[a:b, :]], outs=[m_full[ga:gb, :]])

    def ag_full(m_own, m_full):
        if AG_CHUNKED:
            return
        for k in range(4):
            a = CH_STARTS[k] * TILE_N
            b = a + CH_ROWS[k]
            ga = CH_BASE[k]
            gb = ga + N_CORES * CH_ROWS[k]
            nc.gpsimd.collective_compute(
                "AllGather", AL.bypass, replica_groups=RG,
                ins=[m_own[a:b, :]], outs=[m_full[ga:gb, :]])

    with tile.TileContext(nc) as tc:
        with tc.tile_pool(name="const", bufs=1) as constp, \
             tc.tile_pool(name="big", bufs=1) as bigp, \
             tc.tile_pool(name="xstream", bufs=2) as xp, \
             tc.tile_pool(name="work", bufs=3) as wp, \
             tc.tile_pool(name="gpool", bufs=4) as gp, \
             tc.tile_pool(name="selp", bufs=3) as selp, \
             tc.tile_pool(name="psA", bufs=2, space="PSUM") as psA, \
             tc.tile_pool(name="psT", bufs=2, space="PSUM") as psT, \
             tc.tile_pool(name="psmm", bufs=2, space="PSUM") as psmm:

            # ---- resident constants ----
            w1_t = constp.tile([128, KC1, F1], mybir.dt.bfloat16)
            nc.sync.dma_start(w1_t[:], inp['W1'].rearrange("(kc p) n -> p kc n", p=128))
            w2_t = constp.tile([128, 2, F2], mybir.dt.bfloat16)
            nc.sync.dma_start(w2_t[:], inp['W2'].rearrange("(kc p) n -> p kc n", p=128))
            w3_t = constp.tile([F2, F3], mybir.dt.bfloat16)
            nc.sync.dma_start(w3_t[:], inp['W3'][:, :])
            idx_t = constp.tile([128, SLOTS // 16], mybir.dt.int16)
            nc.sync.dma_start(idx_t[:], inp['idx'][:, :])
            b1bc_t = constp.tile([128, F1], mybir.dt.float32)
            nc.sync.dma_start(b1bc_t[:], inp['b1bc'][:, :])
            ident_t = constp.tile([128, 128], mybir.dt.bfloat16)
            nc.sync.dma_start(ident_t[:], inp['ident'][:, :])

            cdrep_t = constp.tile([128, ROWS_PER_CORE], mybir.dt.float32)
            nc.sync.dma_start(cdrep_t[:], inp['cdst_rep'][:, :])
            cdpp_t = constp.tile([128, TILES_PER_CORE], mybir.dt.float32)
            nc.sync.dma_start(cdpp_t[:], inp['cdst_pp'][:, :])
            cs_t = constp.tile([128, TILES_PER_CORE], mybir.dt.float32)
            nc.sync.dma_start(cs_t[:], inp['csrc_t'][:, :])
            b1_t = constp.tile([128, 2], mybir.dt.float32)
            nc.sync.dma_start(b1_t[:], inp['b1pp'][:, :])
            b2_t = constp.tile([128, 1], mybir.dt.float32)
            nc.sync.dma_start(b2_t[:], inp['b2pp'][:, :])
            b3_t = constp.tile([128, F3], mybir.dt.float32)
            nc.sync.dma_start(b3_t[:], inp['b3t'][:, :])

            h1t = bigp.tile([128, 2, ROWS_PER_CORE], mybir.dt.bfloat16)  # H1.T
            h2t = bigp.tile([F2, ROWS_PER_CORE], mybir.dt.bfloat16)      # H2.T

            # ---- phase 1: M1 = (X @ W1) * c_src, AG1 chunks interleaved ----
            blocks = [(i * 512, 512) for i in range(12)] + [(6144, 128)]
            for c0, bs in blocks:
                xt = xp.tile([128, KC1, bs], mybir.dt.bfloat16, tag="xt")
                nc.sync.dma_start(
                    xt[:, :, :bs],
                    inp['xT'][:, c0:c0 + bs].rearrange("(kc p) n -> p kc n", p=128))
                for sub in range(bs // 128):
                    t_idx = (c0 + sub * 128) // 128
                    ps = psmm.tile([128, F1], mybir.dt.float32, tag="mm1")
                    for kc in range(KC1):
                        nc.tensor.matmul(ps[:], xt[:, kc, sub * 128:(sub + 1) * 128],
                                         w1_t[:, kc, :], start=(kc == 0), stop=(kc == KC1 - 1))
                    ob = wp.tile([128, F1], mybir.dt.float8e4, tag="m1o")
                    nc.vector.tensor_scalar(ob[:], ps[:], cs_t[:, t_idx:t_idx + 1], None, AL.mult)
                    nc.sync.dma_start(m1_own[t_idx * 128:(t_idx + 1) * 128, :], ob[:])
                    if t_idx in CH_ENDS:
                        ag_chunk(m1_own, m1_full, CH_ENDS.index(t_idx))

            ag_full(m1_own, m1_full)

            # ---- agg helper ----
            JPC = CALL // 128   # chunks per gather call
            def agg_layer(m_full_t, elem, consume_chunk, finish_tile):
                cur = {}
                for call in range(N_CALLS):
                    g = gp.tile([128, JPC, elem], mybir.dt.float8e4, tag=f"g{elem}")
                    nc.gpsimd.dma_gather(
                        g[:], m_full_t[IDX_OFF:, :],
                        idx_t[:, call * (CALL // 16):(call + 1) * (CALL // 16)],
                        CALL, CALL, elem, queue_num=call % 4)
                    selt = selp.tile([128, CALL], mybir.dt.float8e4, tag="selt")
                    nc.sync.dma_start(selt[:], inp['selc'][:, call * CALL:(call + 1) * CALL])
                    for j in range(JPC):
                        ch = call * JPC + j
                        t_idx = ch // R_CHUNKS
                        cj = ch % R_CHUNKS
                        consume_chunk(cur, g, j, selt[:, j * 128:(j + 1) * 128], t_idx,
                                      cj == 0, cj == R_CHUNKS - 1)
                        if cj == R_CHUNKS - 1:
                            finish_tile(cur, t_idx)
                            cur = {}

            # ---- layer 1 aggregation -> H1T, M2 + AG2 chunks inline ----
            def l1_chunk(cur, g, j, sel, t_idx, first, last):
                if first:
                    cur[0] = psA.tile([128, F1], mybir.dt.float32, tag="aggA", name="psa1")
                nc.tensor.matmul(cur[0][:], sel, g[:, j, :], start=first, stop=last)

            def l1_tile(cur, t_idx):
                sl = slice(t_idx * 128, (t_idx + 1) * 128)
                hd = wp.tile([128, F1], mybir.dt.bfloat16, tag="hd", name="hd")
                nc.vector.tensor_scalar(hd[:], cur[0][:], cdpp_t[:, t_idx:t_idx + 1], None, AL.mult)
                nc.vector.tensor_tensor(hd[:], hd[:], b1bc_t[:], AL.add)
                nc.scalar.activation(hd[:], hd[:], AF.Relu)
                for fc in range(2):
                    pst = psT.tile([128, 128], mybir.dt.bfloat16, tag="pst", name="pst")
                    nc.tensor.transpose(pst[:], hd[:, fc * 128:(fc + 1) * 128], ident_t[:])
                    nc.scalar.copy(h1t[:, fc, sl], pst[:])
                # M2 tile inline
                ps = psmm.tile([128, F2], mybir.dt.float32, tag="mm1")
                for fc in range(2):
                    nc.tensor.matmul(ps[:], h1t[:, fc, sl], w2_t[:, fc, :],
                                     start=(fc == 0), stop=(fc == 1))
                ob = wp.tile([128, FPAD], mybir.dt.float8e4, tag="m2o")
                nc.vector.tensor_scalar(ob[:, 0:F2], ps[:], cs_t[:, t_idx:t_idx + 1], None, AL.mult)
                nc.sync.dma_start(m2_own[t_idx * 128:(t_idx + 1) * 128, :], ob[:])
                if t_idx in CH_ENDS:
                    ag_chunk(m2_own, m2_full, CH_ENDS.index(t_idx))

            agg_layer(m1_full, F1, l1_chunk, l1_tile)
            ag_full(m2_own, m2_full)

            # ---- layer 2 aggregation -> H2T, M3 + AG3 chunks inline ----
            def l2_chunk(cur, g, j, sel, t_idx, first, last):
                if first:
                    cur[0] = psA.tile([F2, 128], mybir.dt.float32, tag="aggA", name="psa2")
                nc.tensor.matmul(cur[0][:], g[:, j, 0:F2], sel, start=first, stop=last)

            def l2_tile(cur, t_idx):
                sl = slice(t_idx * 128, (t_idx + 1) * 128)
                nc.vector.tensor_tensor(h2t[:, sl], cur[0][:], cdrep_t[0:F2, sl], AL.mult)
                nc.scalar.activation(h2t[:, sl], h2t[:, sl], AF.Relu, bias=b2_t[0:F2, 0:1])
                # M3 tile inline
                ps = psmm.tile([128, F3], mybir.dt.float32, tag="mm1")
                nc.tensor.matmul(ps[:], h2t[:, sl], w3_t[:], start=True, stop=True)
                ob = wp.tile([128, FPAD], mybir.dt.float8e4, tag="m3o")
                nc.vector.tensor_scalar(ob[:, 0:F3], ps[:], cs_t[:, t_idx:t_idx + 1], None, AL.mult)
                nc.sync.dma_start(m3_own[t_idx * 128:(t_idx + 1) * 128, :], ob[:])
                if t_idx in CH_ENDS:
                    ag_chunk(m3_own, m3_full, CH_ENDS.index(t_idx))

            agg_layer(m2_full, FPAD, l2_chunk, l2_tile)
            ag_full(m3_own, m3_full)

            # ---- layer 3 aggregation -> per-tile log_softmax -> out ----
            def l3_chunk(cur, g, j, sel, t_idx, first, last):
                if first:
                    cur[0] = psA.tile([128, F3], mybir.dt.float32, tag="aggA", name="psa3")
                nc.tensor.matmul(cur[0][:], sel, g[:, j, 0:F3], start=first, stop=last)

            def l3_tile(cur, t_idx):
                x3 = wp.tile([128, F3], mybir.dt.float32, tag="x3", name="x3")
                nc.vector.tensor_scalar(x3[:], cur[0][:], cdpp_t[:, t_idx:t_idx + 1], None, AL.mult)
                nc.vector.tensor_tensor(x3[:], x3[:], b3_t[:], AL.add)
                ex = wp.tile([128, F3], mybir.dt.float32, tag="ex", name="ex")
                nc.scalar.activation(ex[:], x3[:], AF.Exp)
                sm = wp.tile([128, 1], mybir.dt.float32, tag="sm", name="sm")
                nc.vector.tensor_reduce(
                    sm[:], ex[:].rearrange("p (t f) -> p t f", f=F3),
                    mybir.AxisListType.X, AL.add)
                nl = wp.tile([128, 1], mybir.dt.float32, tag="nl", name="nl")
                nc.scalar.activation(nl[:], sm[:], AF.Ln)
                ox = wp.tile([128, F3], mybir.dt.float32, tag="ox", name="ox")
                nc.vector.tensor_scalar(ox[:], x3[:], nl[:, 0:1], None, AL.subtract)
                nc.sync.dma_start(out_t[t_idx * 128:(t_idx + 1) * 128, :], ox[:])

            agg_layer(m3_full, FPAD, l3_chunk, l3_tile)

    nc.compile()
    return nc


def _install_profile_shim():
    """Provide the missing antenv.axon_hooks module so trace=True works under axon."""
    try:
        import types
        import antenv
        if 'antenv.axon_hooks' in sys.modules:
            return
        _hook = [None]
        mod = types.ModuleType('antenv.axon_hooks')
        mod.set_axon_ntff_profile_hook = lambda h: _hook.__setitem__(0, h)
        mod.get_axon_ntff_profile_hook = lambda: _hook[0]
        sys.modules['antenv.axon_hooks'] = mod
        antenv.axon_hooks = mod
        from trn_agent_boot.trn_boot import _ntff_profile_via_ctypes
        mod.set_axon_ntff_profile_hook(
            _ntff_profile_via_ctypes('/opt/axon/libaxon_pjrt.so'))
    except Exception:
        pass


_CACHE = {}


def kernel(features, edge_index, W1, b1, W2, b2, W3, b3):
    global last_exec_time_ns
    features = np.asarray(features, dtype=np.float32)
    pre = _preprocess(np.asarray(edge_index))

    if 'nc' not in _CACHE:
        _CACHE['nc'] = _build_nc()
    nc = _CACHE['nc']

    # host-side input prep
    W1p = np.zeros((F_IN_P, F1), dtype=BF16)
    W1p[:F_IN] = np.asarray(W1, dtype=BF16)
    W2b = np.asarray(W2, dtype=BF16)
    W3b = np.asarray(W3, dtype=BF16)
    b1pp = np.asarray(b1, dtype=np.float32).reshape(2, 128).T.copy()
    b2pp = np.zeros((128, 1), dtype=np.float32)
    b2pp[:F2, 0] = np.asarray(b2, dtype=np.float32)
    b3t = np.tile(np.asarray(b3, dtype=np.float32), (128, 1))
    b1bc = np.tile(np.asarray(b1, dtype=np.float32), (128, 1))
    ident = np.eye(128, dtype=BF16)

    # features, permuted and transposed per core: [F_IN_P, 6272] bf16
    feat_b = features.astype(BF16)
    in_maps = []
    for c in range(N_CORES):
        rows = pre['own_node'][c]
        xTc = np.zeros((F_IN_P, ROWS_PER_CORE), dtype=BF16)
        real = rows >= 0
        xTc[:F_IN, real] = feat_b[rows[real]].T
        in_maps.append({
            'xT': xTc, 'W1': W1p, 'W2': W2b, 'W3': W3b,
            'idx': pre['idx_tile'][c], 'selc': pre['selc'][c], 'b1bc': b1bc,
            'ident': ident,
            'cdst_rep': pre['cdst_rep'][c], 'cdst_pp': pre['cdst_pp'][c],
            'csrc_t': pre['csrc_t'][c],
            'b1pp': b1pp, 'b2pp': b2pp, 'b3t': b3t,
        })

    trace = os.environ.get('BASS_KERNEL_TRACE', '0') == '1'
    if trace:
        _install_profile_shim()
    res = run_bass_kernel_spmd(nc, in_maps, core_ids=list(range(N_CORES)), trace=trace)
    last_exec_time_ns = res.exec_time_ns

    # assemble + inverse permute (own rows are tile-major per core)
    out = np.empty((N_NODES, F3), dtype=np.float32)
    for c in range(N_CORES):
        rows = pre['own_node'][c]
        real = rows >= 0
        out[rows[real]] = res.results[c]['out'][real]
    return out
